# revision 1
# baseline (speedup 1.0000x reference)
"""Multi-head cross-attention TRN2 kernel.

N=4096, D=256, H=4, K=16. Data-parallel over 8 NeuronCores: each core owns
512 query rows, key_value + weights replicated. No collectives.

Math (per core, rows R=512):
  QT_h [16,R]   = Wq_h.T @ q^T           (q^T via DMA transpose)
  KhT_h [16,N]  = Wk_h.T @ kv^T          (kv^T via DMA transpose)
  V_aug [N,68]  = kv @ Wv_aug            (per-head 17-col groups: 16 V cols + ones col)
  per head h, per key-chunk m (128 keys):
    S^T[m,:] (PSUM) = KhT_h[:,m].T @ QT_h   -> exp(0.25*S^T) on ACT -> bf16
    heads_psum[17,R] += V_aug[m, h-group].T @ expS^T[m,:]  (row 16 = sum of exp = denom)
  headsTn[16h:,R] = heads_psum[0:16] * bcast(1/heads_psum[16])
  out[R,256] = headsTn.T @ W_o

Matmul operands must sit at base partition 0/32/64 (96 = quadrant-3 bug), and
lhsT/rhs bases must match; so heads are packed two per tile at bases {0,32}:
tile A holds heads 0,1; tile B holds heads 2,3 (both for QT and KhT).

Everything fed to the PE is bf16 (cast on host); accumulation fp32; output fp32.
Measured end-to-end absmax-relative error vs fp32 reference: ~4e-3.
"""
import numpy as np
import ml_dtypes

import concourse.bass as bass
from concourse import bacc
import concourse.mybir as mybir
import concourse.tile as tile
from concourse.bass_utils import run_bass_kernel_spmd

N, D, H, K = 4096, 256, 4, 16
NCORES = 8
R = N // NCORES          # 512 query rows per core
G = K + 1                # 17: per-head V columns + ones column
F32 = mybir.dt.float32
BF16 = mybir.dt.bfloat16
EXPF = mybir.ActivationFunctionType.Exp
BF = ml_dtypes.bfloat16

TRACE = False
LAST_RESULTS = None


def _build(repeats=1):
    nc = bacc.Bacc()
    q = nc.declare_dram_parameter("q", [R, D], BF16, isOutput=False)
    kv = nc.declare_dram_parameter("kv", [N, D], BF16, isOutput=False)
    # wqkv blob: [wq_pad(128) | wk_pad(128) | wv_aug(68)] = 324 cols per d-row;
    # wq/wk padded: head h at cols 64*(h//2)+32*(h%2) .. +16, zeros between.
    wqkv = nc.declare_dram_parameter("wqkv", [656, 128], BF16, isOutput=False)
    # wo blob: [17, 4*256], head h at cols 256h..; row 0 = zeros
    wo = nc.declare_dram_parameter("wo", [G, H * D], BF16, isOutput=False)
    out = nc.declare_dram_parameter("out", [R, D], F32, isOutput=True)

    with tile.TileContext(nc) as tc:
        with (
            tc.tile_pool(name="consts", bufs=1) as consts,
            tc.tile_pool(name="es", bufs=6) as espool,
            tc.tile_pool(name="sbops", bufs=3) as sbops,
            tc.tile_pool(name="spsum", bufs=2, space="PSUM") as spsum,
            tc.tile_pool(name="hpsum", bufs=2, space="PSUM") as hpsum,
            tc.tile_pool(name="mpsum", bufs=2, space="PSUM") as mpsum,
        ):
            for _rep in range(repeats):
                # ---- weights into SBUF: two blob DMAs ----
                # wqkv_sb cols: d-chunk c at 324c: [wq 0:128 | wk 128:256 | wv 256:324]
                wqkv_sb = consts.tile([128, 656], BF16, tag="wqkv_sb", name="wqkv_sb")
                nc.sync.dma_start(out=wqkv_sb, in_=wqkv[:, :], transpose=True)
                ones17 = consts.tile([1, G], BF16, tag="ones17", name="ones17")
                nc.vector.memset(ones17, 1.0)

                # ---- transposed activations via DMA transpose ----
                qt0 = consts.tile([128, R], BF16, tag="qt0", name="qt0")
                qt1 = consts.tile([128, R], BF16, tag="qt1", name="qt1")
                kt0 = consts.tile([128, N], BF16, tag="kt0", name="kt0")
                kt1 = consts.tile([128, N], BF16, tag="kt1", name="kt1")
                nc.sync.dma_start(out=qt0, in_=q[:, 0:128], transpose=True)
                nc.sync.dma_start(out=qt1, in_=q[:, 128:256], transpose=True)
                for j in range(0, N // 512):
                    sl = slice(512 * j, 512 * (j + 1))
                    nc.sync.dma_start(out=kt0[:, sl], in_=kv[sl, 0:128], transpose=True)
                    nc.sync.dma_start(out=kt1[:, sl], in_=kv[sl, 128:256], transpose=True)
                wo_all = consts.tile([G, H * D], BF16, tag="wo_all", name="wo_all")
                nc.sync.dma_start(out=wo_all, in_=wo[:, :])

                # ---- QT tiles: A = heads 0,1 (bases 0,32), B = heads 2,3 ----
                qt_sb = [consts.tile([64, R], BF16, tag=f"qt_sb{t}", name=f"qt_sb{t}")
                         for t in range(2)]
                qt_psum = mpsum.tile([128, R], F32, tag="m", name="m")
                nc.tensor.matmul(qt_psum[:], wqkv_sb[:, 0:128], qt0[:], start=True, stop=False)
                nc.tensor.matmul(qt_psum[:], wqkv_sb[:, 324:452], qt1[:], start=False, stop=True)
                nc.vector.tensor_copy(qt_sb[0][:], qt_psum[0:64, :])
                nc.vector.tensor_copy(qt_sb[1][:], qt_psum[64:128, :])

                # ---- KhT tiles + V_aug, interleaved in consumption order ----
                kht = [consts.tile([64, N], BF16, tag=f"kht{t}", name=f"kht{t}") for t in range(2)]
                v_aug = consts.tile([128, 32 * H * G], BF16, tag="v_aug", name="v_aug")
                # ones columns (pos 0 within each 17-col head group)
                v_ones = v_aug[:].rearrange("p (i g s) -> p i g s", g=H, s=G)[:, :, :, 0:1]
                nc.vector.memset(v_ones, 1.0)
                for j in range(N // 512):
                    kh_psum = mpsum.tile([128, 512], F32, tag="m", name="m")
                    nc.tensor.matmul(kh_psum[:], wqkv_sb[:, 128:256],
                                     kt0[:, 512 * j:512 * (j + 1)], start=True, stop=False)
                    nc.tensor.matmul(kh_psum[:], wqkv_sb[:, 452:580],
                                     kt1[:, 512 * j:512 * (j + 1)], start=False, stop=True)
                    nc.vector.tensor_copy(kht[0][:, 512 * j:512 * (j + 1)], kh_psum[0:64, :])
                    nc.vector.tensor_copy(kht[1][:, 512 * j:512 * (j + 1)], kh_psum[64:128, :])
                    for i in range(4 * j, 4 * j + 4):
                        v_psum = mpsum.tile([128, H * G], F32, tag="m", name="m")
                        nc.tensor.matmul(v_psum[:], kt0[:, 128 * i:128 * (i + 1)],
                                         wqkv_sb[:, 256:324], start=True, stop=False)
                        nc.tensor.matmul(v_psum[:], kt1[:, 128 * i:128 * (i + 1)],
                                         wqkv_sb[:, 580:648], start=False, stop=True)
                        # copy only the 16 V columns of each head group (skip ones col)
                        vsrc = v_psum[:].rearrange("p (g s) -> p g s", s=G)[:, :, 1:G]
                        vdst = v_aug[:, 68 * i:68 * (i + 1)].rearrange(
                            "p (g s) -> p g s", s=G)[:, :, 1:G]
                        nc.vector.tensor_copy(vdst, vsrc)

                # ---- attention, per head, software-pipelined (PE 1 pair ahead of ACT) ----
                # o_acc accumulates W_o partial products across heads (fp32 in SBUF)
                o_acc = consts.tile([128, 4 * D], F32, tag="o_acc", name="o_acc")
                NPAIR = N // 256  # 16 pairs of 128-key chunks
                for h in range(H):
                    t, b = h // 2, 32 * (h % 2)
                    kht_t, qt_t = kht[t], qt_sb[t]
                    heads_psum = hpsum.tile([G, R], F32, tag="heads", name="heads")
                    es_tiles = {}

                    def s_stage(p, kht_t=kht_t, qt_t=qt_t, b=b, es_tiles=es_tiles):
                        s_psum = spsum.tile([128, 1024], F32, tag="s", name="s")
                        lo, hi = 256 * p, 256 * p + 128
                        nc.tensor.matmul(s_psum[:, 0:512],
                                         kht_t[b:b + 16, lo:lo + 128],
                                         qt_t[b:b + 16, :], start=True, stop=True)
                        nc.tensor.matmul(s_psum[:, 512:1024],
                                         kht_t[b:b + 16, hi:hi + 128],
                                         qt_t[b:b + 16, :], start=True, stop=True)
                        es = espool.tile([128, 1024], BF16, tag="es", name="es")
                        nc.scalar.activation(es[:], s_psum[:], EXPF, scale=0.25)
                        es_tiles[p] = es

                    def av_stage(p, heads_psum=heads_psum, es_tiles=es_tiles, h=h):
                        es = es_tiles.pop(p)
                        c0, c1 = 2 * p, 2 * p + 1
                        nc.tensor.matmul(heads_psum[:],
                                         v_aug[:, 68 * c0 + 17 * h:68 * c0 + 17 * h + 17],
                                         es[:, 0:512], start=(p == 0), stop=False)
                        nc.tensor.matmul(heads_psum[:],
                                         v_aug[:, 68 * c1 + 17 * h:68 * c1 + 17 * h + 17],
                                         es[:, 512:1024], start=False, stop=(p == NPAIR - 1))

                    for p in range(NPAIR + 1):
                        if p < NPAIR:
                            s_stage(p)
                        if p >= 1:
                            av_stage(p - 1)

                    # normalize: all 17 rows scaled by 1/row0 (denominator row)
                    recip = sbops.tile([1, R], F32, tag="recip", name="recip")
                    nc.vector.reciprocal(recip[:], heads_psum[0:1, :])
                    recipb = sbops.tile([1, R], BF16, tag="recipb", name="recipb")
                    nc.vector.tensor_copy(recipb[:], recip[:])
                    rb_psum = mpsum.tile([G, R], F32, tag="m", name="m")
                    nc.tensor.matmul(rb_psum[:], ones17[:], recipb[:], start=True, stop=True)
                    headsT_sb = sbops.tile([G, R], F32, tag="headsT_sb", name="headsT_sb")
                    nc.vector.tensor_copy(headsT_sb[:], heads_psum[:])
                    # per-chunk: normalize -> W_o partial -> accumulate -> (last head) store
                    for c in range(R // 128):
                        cs = slice(128 * c, 128 * (c + 1))
                        hn_c = sbops.tile([G, 128], BF16, tag="hn", name="hn")
                        nc.vector.tensor_mul(hn_c[:], headsT_sb[:, cs], rb_psum[:, cs])
                        o_psum = mpsum.tile([128, D], F32, tag="m", name="m")
                        nc.tensor.matmul(o_psum[:], hn_c[:],
                                         wo_all[:, D * h:D * (h + 1)], start=True, stop=True)
                        osl = o_acc[:, D * c:D * (c + 1)]
                        if h == 0:
                            nc.vector.tensor_copy(osl, o_psum[:])
                        else:
                            nc.vector.tensor_add(osl, osl, o_psum[:])
                        if h == H - 1:
                            nc.sync.dma_start(out=out[cs, :], in_=osl)

    nc.finalize()
    return nc


_NC_CACHE = None


def _host_in_maps(query, key_value, W_q, W_k, W_v, W_o):
    q_bf = np.ascontiguousarray(query.astype(BF))
    kv_bf = np.ascontiguousarray(key_value.astype(BF))
    # padded wq/wk: head h at cols 64*(h//2)+32*(h%2) .. +16
    wqkv_h = np.zeros((D, 324), dtype=BF)
    wqt = np.transpose(W_q, (1, 0, 2))  # [D, H, K]
    wkt = np.transpose(W_k, (1, 0, 2))
    wvt = np.transpose(W_v, (1, 0, 2))
    for h in range(H):
        c0 = 64 * (h // 2) + 32 * (h % 2)
        wqkv_h[:, c0:c0 + K] = wqt[:, h, :].astype(BF)
        wqkv_h[:, 128 + c0:128 + c0 + K] = wkt[:, h, :].astype(BF)
        wqkv_h[:, 256 + G * h + 1:256 + G * (h + 1)] = wvt[:, h, :].astype(BF)
    wqkv_h = np.ascontiguousarray(np.concatenate(
        [wqkv_h[0:128].T, wqkv_h[128:256].T, np.zeros((8, 128), dtype=BF)], axis=0))
    wo_h = np.zeros((G, H * D), dtype=BF)
    wo_r = W_o.reshape(H, K, D)
    for h in range(H):
        wo_h[1:G, D * h:D * (h + 1)] = wo_r[h].astype(BF)
    return [{"q": q_bf[c * R:(c + 1) * R], "kv": kv_bf, "wqkv": wqkv_h, "wo": wo_h}
            for c in range(NCORES)]


def kernel(query, key_value, W_q, W_k, W_v, W_o):
    global _NC_CACHE, LAST_RESULTS
    if _NC_CACHE is None:
        _NC_CACHE = _build()
    nc = _NC_CACHE
    in_maps = _host_in_maps(query, key_value, W_q, W_k, W_v, W_o)
    res = run_bass_kernel_spmd(nc, in_maps, list(range(NCORES)), trace=TRACE)
    LAST_RESULTS = res
    return np.concatenate([res.results[c]["out"] for c in range(NCORES)], axis=0)



# revision 9
# speedup vs baseline: 60.7820x; 60.7820x over previous
"""Multi-head cross-attention TRN2 kernel (v2).

N=4096, D=256, H=4, K=16. Data-parallel over 8 NeuronCores: each core owns
512 query rows; key_value + weights replicated. No collectives.

Key ideas vs baseline:
- Host pre-transposes q/kv (layout prep only), so no device DMA transposes.
- Projections pack all 4 heads at partition offsets 32h, so the S matmuls
  run 4-way concurrent via PE row tiling (tile_position=(32h,0)) and the AV
  matmuls run 4-way concurrent via col tiling (tile_position=(0,32h)).
- The softmax exp (the elementwise wall: 65536 psum->sbuf elems/lane) is
  split across BOTH ScalarE (true exp) and VectorE (Schraudolph bit-trick:
  bf16 bits of exp(x) ~= int16(x*128*log2e + 16250.5), one tensor_scalar).
- W_q is pre-scaled by 0.25 (the 1/sqrt(K) softmax scale) on host.
- Denominators ride along as a ones-column in v_aug (row 32h of AV psum);
  normalization is recip + PE broadcast + one tensor_mul.
"""
import numpy as np
import ml_dtypes

import concourse.bass as bass
from concourse import bacc
import concourse.mybir as mybir
import concourse.tile as tile
from concourse.bass_utils import run_bass_kernel_spmd

N, D, H, K = 4096, 256, 4, 16
NCORES = 8
R = N // NCORES          # 512 query rows per core
G = K + 1                # 17: ones column + 16 V dims per head group
NKC = N // 128           # 32 key chunks
F32 = mybir.dt.float32
BF16 = mybir.dt.bfloat16
I16 = mybir.dt.int16
EXPF = mybir.ActivationFunctionType.Exp
MULT = mybir.AluOpType.mult
ADD = mybir.AluOpType.add
BF = ml_dtypes.bfloat16

SCH_MULT = float(128.0 / np.log(2.0))   # 184.664
SCH_BIAS = 16256.0 - 5.5                # Schraudolph magic for bf16 bits

TRACE = False
LAST_RESULTS = None


def _build(repeats=1):
    nc = bacc.Bacc()
    qt_d = nc.declare_dram_parameter("qt", [D, R], BF16, isOutput=False)
    kvt_d = nc.declare_dram_parameter("kvt", [D, N], BF16, isOutput=False)
    wq_d = nc.declare_dram_parameter("wq", [D, 128], BF16, isOutput=False)
    wk_d = nc.declare_dram_parameter("wk", [D, 128], BF16, isOutput=False)
    wv_d = nc.declare_dram_parameter("wv", [D, 68], BF16, isOutput=False)
    wo_d = nc.declare_dram_parameter("wo", [128, D], BF16, isOutput=False)
    on4_d = nc.declare_dram_parameter("on4", [128, 128], BF16, isOutput=False)
    out_d = nc.declare_dram_parameter("out", [R, D], F32, isOutput=True)

    with tile.TileContext(nc) as tc:
        with (
            tc.tile_pool(name="consts", bufs=1) as consts,
            tc.tile_pool(name="es", bufs=3) as espool,
            tc.tile_pool(name="sbops", bufs=2) as sbops,
            tc.tile_pool(name="sp", bufs=2, space="PSUM") as spool,
            tc.tile_pool(name="avp", bufs=1, space="PSUM") as avpool,
            tc.tile_pool(name="mp", bufs=1, space="PSUM") as mpool,
        ):
            for _rep in range(repeats):
                # ---------- DMA in ----------
                wq_sb = consts.tile([128, 256], BF16, tag="wq", name="wq")
                wk_sb = consts.tile([128, 256], BF16, tag="wk", name="wk")
                wv_sb = consts.tile([128, 136], BF16, tag="wv", name="wv")
                wo_sb = consts.tile([128, 256], BF16, tag="wo", name="wo")
                on4_sb = consts.tile([128, 128], BF16, tag="on4", name="on4")
                for half in range(2):
                    dsl = slice(128 * half, 128 * (half + 1))
                    nc.sync.dma_start(out=wq_sb[:, 128 * half:128 * half + 128],
                                      in_=wq_d[dsl, :])
                    nc.sync.dma_start(out=wk_sb[:, 128 * half:128 * half + 128],
                                      in_=wk_d[dsl, :])
                    nc.sync.dma_start(out=wv_sb[:, 68 * half:68 * half + 68],
                                      in_=wv_d[dsl, :])
                nc.sync.dma_start(out=wo_sb, in_=wo_d[:, :])
                nc.sync.dma_start(out=on4_sb, in_=on4_d[:, :])

                qt_raw = consts.tile([128, 1024], BF16, tag="qtr", name="qtr")
                nc.sync.dma_start(out=qt_raw[:, 0:512], in_=qt_d[0:128, :])
                nc.sync.dma_start(out=qt_raw[:, 512:1024], in_=qt_d[128:256, :])

                kt = consts.tile([128, 8192], BF16, tag="kt", name="kt")
                for j in range(8):
                    tsl = slice(512 * j, 512 * (j + 1))
                    nc.sync.dma_start(out=kt[:, 512 * j:512 * (j + 1)],
                                      in_=kvt_d[0:128, tsl])
                    nc.sync.dma_start(out=kt[:, 4096 + 512 * j:4096 + 512 * (j + 1)],
                                      in_=kvt_d[128:256, tsl])

                # ---------- persistent SBUF results ----------
                kht = consts.tile([128, N], BF16, tag="kht", name="kht")
                v_aug = consts.tile([128, NKC * 68], BF16, tag="v_aug", name="v_aug")
                qt_sb = consts.tile([128, 512], BF16, tag="qt_sb", name="qt_sb")
                hn_sb = consts.tile([128, 512], BF16, tag="hn_sb", name="hn_sb")

                # ones slots of v_aug (col 68c + 17h), written once on gpsimd
                von = v_aug[:].rearrange("p (i h s) -> p i h s", i=NKC, h=H, s=G)[:, :, :, 0:1]
                nc.gpsimd.memset(von, 1.0)

                # AV accumulator: zero data so never-written rows stay finite
                av_ps = avpool.tile([128, 512], F32, tag="av", name="av")
                nc.vector.memset(av_ps[:], 0.0)

                # ---------- QT projection ----------
                qt_psum = mpool.tile([128, 512], F32, tag="kh", name="kh")
                nc.tensor.matmul(qt_psum[:], wq_sb[:, 0:128], qt_raw[:, 0:512],
                                 start=True, stop=False)
                nc.tensor.matmul(qt_psum[:], wq_sb[:, 128:256], qt_raw[:, 512:1024],
                                 start=False, stop=True)
                nc.scalar.copy(qt_sb[:], qt_psum[:])

                # ---------- interleaved projections + attention ----------
                def proj(j):
                    kh_psum = mpool.tile([128, 512], F32, tag="kh", name="kh")
                    tsl = slice(512 * j, 512 * (j + 1))
                    nc.tensor.matmul(kh_psum[:], wk_sb[:, 0:128], kt[:, tsl],
                                     start=True, stop=False)
                    nc.tensor.matmul(kh_psum[:], wk_sb[:, 128:256],
                                     kt[:, 4096 + 512 * j:4096 + 512 * (j + 1)],
                                     start=False, stop=True)
                    if j % 2 == 0:
                        nc.scalar.copy(kht[:, tsl], kh_psum[:])
                    else:
                        nc.vector.tensor_copy(kht[:, tsl], kh_psum[:])

                    v_psum = mpool.tile([128, 272], F32, tag="v", name="v")
                    for s in range(4):
                        i = 4 * j + s
                        nc.tensor.matmul(v_psum[:, 68 * s:68 * (s + 1)],
                                         kt[:, 128 * i:128 * (i + 1)],
                                         wv_sb[:, 0:68], start=True, stop=False)
                        nc.tensor.matmul(v_psum[:, 68 * s:68 * (s + 1)],
                                         kt[:, 4096 + 128 * i:4096 + 128 * (i + 1)],
                                         wv_sb[:, 68:136], start=False, stop=True)
                    # copy the 16 V cols of each head group (skip ones col)
                    vsrc = v_psum[:].rearrange("p (s h g) -> p s h g", s=4, g=G)[:, :, :, 1:G]
                    vdst = v_aug[:, 272 * j:272 * (j + 1)].rearrange(
                        "p (s h g) -> p s h g", s=4, g=G)[:, :, :, 1:G]
                    if j % 2 == 0:
                        nc.vector.tensor_copy(vdst, vsrc)
                    else:
                        nc.scalar.copy(vdst, vsrc)

                first_av = [True]

                def attn(c):
                    # S: two head-pair instances; each [128,1024] f32 tile is
                    # exactly 2 PSUM banks, one per concurrent row-tile
                    # (same-bank concurrent row tiles are fatal on TRN2 HW).
                    es_tiles = []
                    for hp in range(2):
                        s_ps = spool.tile([128, 1024], F32, tag="s", name="s")
                        for i in range(2):
                            h = 2 * hp + i
                            nc.tensor.matmul(
                                s_ps[:, 512 * i:512 * (i + 1)],
                                kht[32 * h:32 * h + 16, 128 * c:128 * (c + 1)],
                                qt_sb[32 * h:32 * h + 16, :],
                                start=True, stop=True, tile_position=(32 * h, 0))
                        es = espool.tile([128, 1024], BF16, tag="es", name="es")
                        idx = 2 * c + hp
                        if idx % 15 < 8:
                            nc.scalar.activation(es[:], s_ps[:], EXPF, scale=1.0)
                        else:
                            nc.vector.tensor_scalar(
                                es[:].bitcast(I16), s_ps[:], SCH_MULT, SCH_BIAS,
                                MULT, ADD)
                        es_tiles.append(es)
                    for h in range(H):
                        nc.tensor.matmul(
                            av_ps[32 * h:32 * h + G, :],
                            v_aug[:, 68 * c + 17 * h:68 * c + 17 * h + G],
                            es_tiles[h // 2][:, 512 * (h % 2):512 * (h % 2 + 1)],
                            start=first_av[0], stop=(c == NKC - 1 and h == H - 1),
                            tile_position=(0, 32 * h), skip_group_check=True)
                        first_av[0] = False

                proj(0)
                proj(1)
                for j in range(2, 10):
                    for c in range(4 * (j - 2), 4 * (j - 1)):
                        attn(c)
                    if j < 8:
                        proj(j)

                # ---------- normalize + W_o + out ----------
                # broadcast raw denominators (av_ps row 32h) to every row of
                # head h's 32-block: rb[p, q] = den[p // 32, q], then one
                # full-width approx reciprocal and one multiply.
                av_sb = sbops.tile([128, 512], BF16, tag="av_sb", name="av_sb")
                nc.scalar.copy(av_sb[:], av_ps[:])
                rb_ps = spool.tile([128, 1024], F32, tag="s", name="s")
                nc.tensor.matmul(rb_ps[:, 0:512], on4_sb[:], av_sb[:],
                                 start=True, stop=True)
                recip_sb = sbops.tile([128, 512], F32, tag="recip_sb",
                                      name="recip_sb")
                nc.vector.reciprocal_approx_fast(recip_sb[:], rb_ps[:, 0:512])
                nc.vector.tensor_mul(hn_sb[:], av_ps[:], recip_sb[:])

                for half in range(2):
                    wo_ps = mpool.tile([128, 512], F32, tag="kh", name="kh")
                    for tt in range(2):
                        t = 2 * half + tt
                        nc.tensor.matmul(wo_ps[:, 256 * tt:256 * (tt + 1)],
                                         hn_sb[:, 128 * t:128 * (t + 1)],
                                         wo_sb[:], start=True, stop=True)
                    osb = sbops.tile([128, 512], F32, tag="osb", name="osb")
                    if half == 0:
                        nc.scalar.copy(osb[:], wo_ps[:])
                    else:
                        nc.vector.tensor_copy(osb[:], wo_ps[:])
                    odst = out_d[256 * half:256 * (half + 1), :].rearrange(
                        "(tt p) d -> p tt d", tt=2)
                    nc.sync.dma_start(out=odst, in_=osb[:].rearrange(
                        "p (tt d) -> p tt d", tt=2))

    nc.finalize()
    return nc


_NC_CACHE = None


def _host_in_maps(query, key_value, W_q, W_k, W_v, W_o):
    qt = np.ascontiguousarray(query.astype(BF).T)        # [D, N]
    kvt = np.ascontiguousarray(key_value.astype(BF).T)   # [D, N]
    wq = np.zeros((D, 128), dtype=BF)
    wk = np.zeros((D, 128), dtype=BF)
    wv = np.zeros((D, 68), dtype=BF)
    for h in range(H):
        wq[:, 32 * h:32 * h + K] = (W_q[h] * 0.25).astype(BF)
        wk[:, 32 * h:32 * h + K] = W_k[h].astype(BF)
        wv[:, 17 * h + 1:17 * (h + 1)] = W_v[h].astype(BF)
    wo = np.zeros((128, D), dtype=BF)
    wo_r = W_o.reshape(H, K, D)
    for h in range(H):
        wo[32 * h + 1:32 * h + 1 + K, :] = wo_r[h].astype(BF)
    on4 = np.zeros((128, 128), dtype=BF)
    for g in range(4):
        on4[32 * g, 32 * g:32 * (g + 1)] = 1.0
    return [{"qt": np.ascontiguousarray(qt[:, c * R:(c + 1) * R]), "kvt": kvt,
             "wq": wq, "wk": wk, "wv": wv, "wo": wo, "on4": on4}
            for c in range(NCORES)]


def kernel(query, key_value, W_q, W_k, W_v, W_o):
    global _NC_CACHE, LAST_RESULTS
    if _NC_CACHE is None:
        _NC_CACHE = _build()
    nc = _NC_CACHE
    in_maps = _host_in_maps(query, key_value, W_q, W_k, W_v, W_o)
    res = run_bass_kernel_spmd(nc, in_maps, list(range(NCORES)), trace=TRACE)
    LAST_RESULTS = res
    return np.concatenate([res.results[c]["out"] for c in range(NCORES)], axis=0)
